# revision 2
# baseline (speedup 1.0000x reference)
"""Bahdanau attention kernel for Trainium2 (Bass/Tile), 8-core data-parallel.

Problem shapes: B=32, Tx=1024, enc_hid=dec_hid=attn=1024.

v2: bf16 streams + software-pipelined PE schedule.
  - All big matmul operands in bf16 (inputs host-cast): halves DMA
    (42MB -> ~21MB) and SBUF, enables FWL fast weight loads. Numerically
    validated vs fp32 reference: ctx rel 2.2e-3, alpha 4.7e-3 (tol 2e-2).
  - Host pre-tiles every tensor so each DMA is a contiguous 2D slab with
    2KB+ per-partition lines.
  - Energy loop nt(512-t-halves)-outer so scores for half 0 finish at
    example midpoint: the context matmuls accumulate per half, shrinking
    the end-of-kernel tail.
  - exp computed with ACT accum_out => softmax sum is free.
  - alpha row -> column transpose via 8 tiny PE transposes (no DRAM
    bounce).
  - PE stream software-pipelined: next example's first two energy groups
    are emitted before the current example's softmax tail so the PE
    never waits on the DVE/ACT softmax chain.

Math (per example b):
  dec_proj = W_dec @ dec_hidden[b]                 [attn]
  energy^T[a, t] = tanh(sum_e W_enc[a,e] enc[b,t,e] + dec_proj[a] + W_b[a])
  scores[t] = sum_a v[a] energy^T[a, t]
  alpha = softmax(scores + (mask-1)*50)
  context[e] = sum_t alpha[t] enc[b,t,e]
"""

from contextlib import ExitStack

import numpy as np
import ml_dtypes

import concourse.bass as bass
import concourse.tile as tile
from concourse import bacc, mybir
from concourse.masks import make_identity

F32 = mybir.dt.float32
BF16 = mybir.dt.bfloat16
AF = mybir.ActivationFunctionType
BF = ml_dtypes.bfloat16

P = 128
N_CORES = 8
B_LOC = 4            # examples per core
TX = 1024
E = 1024             # enc_hid
A = 1024             # attn
D = 1024             # dec_hid
EO = E // P          # e-chunks
AO = A // P          # a-chunks
TO = TX // P         # t-chunks
DO = D // P          # d-chunks
NT = 2               # 512-wide t-halves
ET = 2               # 512-wide e-halves


def build_nc():
    nc = bacc.Bacc(
        "TRN2", target_bir_lowering=False, debug=False, num_devices=N_CORES
    )
    # Host-pre-tiled DRAM inputs (all contiguous 2D slabs per DMA).
    encT_d = nc.dram_tensor("encT", [B_LOC, NT, P, EO * 512], BF16, kind="ExternalInput").ap()
    encN_d = nc.dram_tensor("encN", [B_LOC, P, TO * E], BF16, kind="ExternalInput").ap()
    wenc_d = nc.dram_tensor("w_encT", [AO, P, EO * P], BF16, kind="ExternalInput").ap()
    wdec_d = nc.dram_tensor("w_decT", [2, P, DO * 512], BF16, kind="ExternalInput").ap()
    dech_d = nc.dram_tensor("dec_hT", [P, DO * B_LOC], BF16, kind="ExternalInput").ap()
    v_d = nc.dram_tensor("v_col", [P, AO], BF16, kind="ExternalInput").ap()
    wb_d = nc.dram_tensor("wb8", [P, AO], F32, kind="ExternalInput").ap()
    maskb_d = nc.dram_tensor("maskb", [B_LOC, TX], F32, kind="ExternalInput").ap()
    ctx_out = nc.dram_tensor("context", [B_LOC, E], F32, kind="ExternalOutput").ap()
    alpha_out = nc.dram_tensor("alpha", [B_LOC, TX], F32, kind="ExternalOutput").ap()

    with tile.TileContext(nc) as tc, ExitStack() as ctx:
        const = ctx.enter_context(tc.tile_pool(name="const", bufs=1))
        encT_pool = ctx.enter_context(tc.tile_pool(name="encTp", bufs=3))
        encN_pool = ctx.enter_context(tc.tile_pool(name="encNp", bufs=3))
        en_pool = ctx.enter_context(tc.tile_pool(name="energy", bufs=6))
        rowp = ctx.enter_context(tc.tile_pool(name="rows", bufs=2))
        small = ctx.enter_context(tc.tile_pool(name="small", bufs=2))
        # PSUM bank budget (8 banks): ep 2 + sc 2 + cx 2 + tiny 2.
        ps_ep = ctx.enter_context(tc.tile_pool(name="ps_ep", bufs=2, space="PSUM"))
        ps_sc = ctx.enter_context(tc.tile_pool(name="ps_sc", bufs=2, space="PSUM"))
        ps_cx = ctx.enter_context(tc.tile_pool(name="ps_cx", bufs=2, space="PSUM"))
        ps_tiny = ctx.enter_context(tc.tile_pool(name="ps_tiny", bufs=2, space="PSUM"))

        wenc_sb = const.tile([P, AO, EO, P], BF16)
        wdec_sb = const.tile([P, 2, DO, 512], BF16)
        dech_sb = const.tile([P, DO, B_LOC], BF16)
        v_sb = const.tile([P, AO], BF16)
        wb_sb = const.tile([P, AO], F32)
        bias_sb = const.tile([P, AO, B_LOC], F32)
        ident4 = const.tile([B_LOC, B_LOC], F32)
        ident1 = const.tile([1, 1], F32)
        make_identity(nc, ident4[:])
        make_identity(nc, ident1[:])

        # tiny dependency-free loads on gpsimd
        nc.gpsimd.dma_start(dech_sb[:].rearrange("p do b -> p (do b)"), dech_d[:])
        nc.gpsimd.dma_start(v_sb[:], v_d[:])
        nc.gpsimd.dma_start(wb_sb[:], wb_d[:])
        mask_rows = []
        for b in range(B_LOC):
            mr = small.tile([1, TX], F32, tag="mrow", bufs=B_LOC, name=f"mask{b}")
            nc.gpsimd.dma_start(mr[:], maskb_d[b : b + 1, :])
            mask_rows.append(mr)

        # DMA lanes: 3-way round-robin for ungated loads; sync/gpsimd only
        # for gated (recycled-slot) loads so a waiting enqueue never
        # head-of-line blocks the ACT compute stream.
        lanes3 = [nc.sync, nc.scalar, nc.gpsimd]
        lanes2 = [nc.sync, nc.gpsimd]
        li3 = [0]
        li2 = [0]

        def lane3():
            e = lanes3[li3[0] % 3]
            li3[0] += 1
            return e

        def lane2():
            e = lanes2[li2[0] % 2]
            li2[0] += 1
            return e

        def dma_chunks(dst2d, src2d, nchunks, lane_fn):
            n = src2d.shape[-1]
            step = n // nchunks
            for i in range(nchunks):
                lane_fn().dma_start(
                    dst2d[:, i * step : (i + 1) * step],
                    src2d[:, i * step : (i + 1) * step],
                )

        encT_tiles = {}
        encN_tiles = {}

        def alloc_encT(b):
            encT_tiles[b] = encT_pool.tile(
                [P, NT, EO, 512], BF16, tag="encT", name=f"encT{b}"
            )

        def alloc_encN(b):
            encN_tiles[b] = encN_pool.tile(
                [P, TO, E], BF16, tag="encN", name=f"encN{b}"
            )

        def encT_2d(b, nt):
            return encT_tiles[b][:, nt].rearrange("p eo j -> p (eo j)")

        def encN_2d(b):
            return encN_tiles[b][:].rearrange("p to e -> p (to e)")

        def wenc_2d(ao):
            return wenc_sb[:, ao].rearrange("p eo c -> p (eo c)")

        # ---- bulk loads, emitted in NEED order, fine-grained at the start
        # so the three queues deliver the startup-critical tensors in
        # parallel rather than serializing 1MB slabs on one queue.
        alloc_encT(0)
        alloc_encT(1)
        alloc_encT(2)
        alloc_encN(0)
        alloc_encN(1)
        wdec0_2d = wdec_sb[:, 0].rearrange("p do j -> p (do j)")
        wdec1_2d = wdec_sb[:, 1].rearrange("p do j -> p (do j)")
        dma_chunks(wdec0_2d, wdec_d[0], 2, lane3)
        dma_chunks(wenc_2d(0), wenc_d[0], 1, lane3)
        dma_chunks(wenc_2d(1), wenc_d[1], 1, lane3)
        dma_chunks(encT_2d(0, 0), encT_d[0, 0], 2, lane3)
        dma_chunks(wdec1_2d, wdec_d[1], 2, lane3)
        dma_chunks(wenc_2d(2), wenc_d[2], 1, lane3)
        dma_chunks(wenc_2d(3), wenc_d[3], 1, lane3)
        dma_chunks(encT_2d(0, 1), encT_d[0, 1], 2, lane3)
        for ao in range(4, 8):
            dma_chunks(wenc_2d(ao), wenc_d[ao], 1, lane3)
        dma_chunks(encT_2d(1, 0), encT_d[1, 0], 1, lane3)
        dma_chunks(encT_2d(1, 1), encT_d[1, 1], 1, lane3)
        dma_chunks(encN_2d(0), encN_d[0], 2, lane3)
        dma_chunks(encT_2d(2, 0), encT_d[2, 0], 1, lane3)
        dma_chunks(encT_2d(2, 1), encT_d[2, 1], 1, lane3)
        dma_chunks(encN_2d(1), encN_d[1], 2, lane3)

        # ---- per-example state ------------------------------------------
        class Ex:
            pass

        exs = {}

        def get_ex(b):
            if b in exs:
                return exs[b]
            s = Ex()
            s.sc = [
                ps_sc.tile([1, 512], F32, tag="sc", name=f"sc{b}_{nt}")
                for nt in range(NT)
            ]
            s.msc = rowp.tile([1, TX], F32, tag="msc", name=f"msc{b}")
            s.expf = rowp.tile([1, TX], F32, tag="expf", name=f"expf{b}")
            s.s2 = small.tile([1, 2], F32, tag="s2", name=f"s2_{b}")
            s.expT_ps = ps_tiny.tile([P, TO], F32, tag="tiny", name=f"expTps{b}")
            s.expT = small.tile([P, TO], BF16, tag="expT", name=f"expT{b}")
            s.cx = [
                ps_cx.tile([1, 512], F32, tag="cx", name=f"cx{b}_{et}")
                for et in range(ET)
            ]
            s.alpha_row = rowp.tile([1, TX], F32, tag="arow", name=f"alpha{b}")
            s.ctx_row = rowp.tile([1, E], F32, tag="crow", name=f"ctx{b}")
            exs[b] = s
            return s

        # ---- compute building blocks ------------------------------------
        dp_row = rowp.tile([B_LOC, A], F32, tag="dprow", bufs=1)

        def dec_half(h):
            # dec_proj for a-columns [512h, 512h+512), then transpose into
            # bias_sb[a-part, b] and add W_b.
            dp = ps_ep.tile([P, 512], F32, tag="ep", name=f"dp{h}")
            for do in range(DO):
                nc.tensor.matmul(
                    dp[:B_LOC, :],
                    lhsT=dech_sb[:, do],
                    rhs=wdec_sb[:, h, do],
                    start=(do == 0),
                    stop=(do == DO - 1),
                )
            nc.vector.tensor_copy(dp_row[:, h * 512 : (h + 1) * 512], dp[:B_LOC, :])
            for ao in range(4 * h, 4 * h + 4):
                tp = ps_tiny.tile([P, B_LOC], F32, tag="tiny", name=f"tp{ao}")
                nc.tensor.transpose(tp[:], dp_row[:, ao * P : (ao + 1) * P], ident4[:])
                nc.vector.tensor_scalar_add(bias_sb[:, ao], tp[:], wb_sb[:, ao : ao + 1])

        def e_group(b, nt, ao):
            # energy^T tile [128a, 512t] + score accumulation
            s = get_ex(b)
            ep = ps_ep.tile([P, 512], F32, tag="ep", name=f"ep{b}_{nt}_{ao}")
            for eo in range(EO):
                nc.tensor.matmul(
                    ep[:],
                    lhsT=wenc_sb[:, ao, eo],
                    rhs=encT_tiles[b][:, nt, eo],
                    start=(eo == 0),
                    stop=(eo == EO - 1),
                )
            en = en_pool.tile([P, 512], BF16, tag="energy", name=f"en{b}_{nt}_{ao}")
            nc.scalar.activation(en[:], ep[:], AF.Tanh, bias=bias_sb[:, ao, b : b + 1])
            nc.tensor.matmul(
                s.sc[nt][:],
                lhsT=v_sb[:, ao : ao + 1],
                rhs=en[:],
                start=(ao == 0),
                stop=(ao == AO - 1),
            )

        def half_block(b, nt):
            # scores half -> masked -> exp (sum via accum) -> expT columns
            # -> context accumulation over this half's t-chunks.
            s = get_ex(b)
            hs = slice(nt * 512, (nt + 1) * 512)
            nc.vector.tensor_add(s.msc[:, hs], s.sc[nt][:], mask_rows[b][:, hs])
            nc.scalar.activation(
                s.expf[:, hs], s.msc[:, hs], AF.Exp,
                accum_out=s.s2[:, nt : nt + 1],
            )
            for i in range(4):
                to = nt * 4 + i
                nc.tensor.transpose(
                    s.expT_ps[:, to : to + 1],
                    s.expf[:, to * P : (to + 1) * P],
                    ident1[:],
                )
            nc.vector.tensor_copy(
                s.expT[:, nt * 4 : nt * 4 + 4], s.expT_ps[:, nt * 4 : nt * 4 + 4]
            )
            for et in range(ET):
                for i in range(4):
                    to = nt * 4 + i
                    nc.tensor.matmul(
                        s.cx[et][:],
                        lhsT=s.expT[:, to : to + 1],
                        rhs=encN_tiles[b][:, to, et * 512 : (et + 1) * 512],
                        start=(to == 0),
                        stop=(to == TO - 1),
                    )

        def fin(b):
            # softmax normalization (scores are bounded, |s| <= sum|v| ~ 26,
            # so exp needs no max shift) + context evacuation + outputs.
            s = get_ex(b)
            ssum = small.tile([1, 1], F32, tag="ssum", name=f"ssum{b}")
            nc.vector.tensor_add(ssum[:], s.s2[:, 0:1], s.s2[:, 1:2])
            rsum = small.tile([1, 1], F32, tag="rsum", name=f"rsum{b}")
            nc.vector.reciprocal(rsum[:], ssum[:])
            nc.vector.tensor_scalar_mul(s.alpha_row[:], s.expf[:], rsum[:])
            nc.sync.dma_start(alpha_out[b : b + 1, :], s.alpha_row[:])
            for et in range(ET):
                nc.vector.tensor_scalar_mul(
                    s.ctx_row[:, et * 512 : (et + 1) * 512], s.cx[et][:], rsum[:]
                )
            nc.sync.dma_start(ctx_out[b : b + 1, :], s.ctx_row[:])

        # ---- software-pipelined program ---------------------------------
        dec_half(0)
        e_group(0, 0, 0)
        e_group(0, 0, 1)
        dec_half(1)
        for ao in range(2, 8):
            e_group(0, 0, ao)
        e_group(0, 1, 0)
        e_group(0, 1, 1)
        half_block(0, 0)
        for ao in range(2, 8):
            e_group(0, 1, ao)

        for b in range(B_LOC):
            nb = b + 1
            if b == 0:
                alloc_encN(2)
                dma_chunks(encN_2d(2), encN_d[2], 2, lane2)
            if b == 1:
                alloc_encT(3)
                dma_chunks(encT_2d(3, 0), encT_d[3, 0], 1, lane2)
                dma_chunks(encT_2d(3, 1), encT_d[3, 1], 1, lane2)
                alloc_encN(3)
                dma_chunks(encN_2d(3), encN_d[3], 2, lane2)
            if nb < B_LOC:
                # overlap this example's softmax tail with the next
                # example's first energy groups
                e_group(nb, 0, 0)
                e_group(nb, 0, 1)
            half_block(b, 1)
            fin(b)
            if nb < B_LOC:
                for ao in range(2, 8):
                    e_group(nb, 0, ao)
                e_group(nb, 1, 0)
                e_group(nb, 1, 1)
                half_block(nb, 0)
                for ao in range(2, 8):
                    e_group(nb, 1, ao)

    nc.compile()
    return nc


_NC = None


def _get_nc():
    global _NC
    if _NC is None:
        _NC = build_nc()
    return _NC


def make_in_maps(dec_hidden, enc_outputs, mask, W_w, W_b, v_w):
    dec_hidden = np.asarray(dec_hidden, np.float32)
    enc_outputs = np.asarray(enc_outputs, np.float32)
    mask = np.asarray(mask)
    W_w = np.asarray(W_w, np.float32)
    W_b = np.asarray(W_b, np.float32)
    v_w = np.asarray(v_w, np.float32)

    enc16 = enc_outputs.astype(BF)                       # [B, TX, E]
    # encT[b, nt, p, eo*512+j] = enc[b, nt*512+j, eo*128+p]
    x = enc16.reshape(-1, NT, 512, EO, P)
    encT = np.ascontiguousarray(x.transpose(0, 1, 4, 3, 2)).reshape(
        -1, NT, P, EO * 512
    )
    # encN[b, p, to*1024+e] = enc[b, to*128+p, e]
    y = enc16.reshape(-1, TO, P, E)
    encN = np.ascontiguousarray(y.transpose(0, 2, 1, 3)).reshape(-1, P, TO * E)

    W_enc = W_w[:, D:].astype(BF)                        # [A, E]
    wencT = np.ascontiguousarray(
        W_enc.reshape(AO, P, EO, P).transpose(0, 3, 2, 1)
    ).reshape(AO, P, EO * P)
    W_dec = W_w[:, :D].astype(BF)                        # [A, D]
    wdecT = np.ascontiguousarray(
        W_dec.T.reshape(DO, P, 2, 512).transpose(2, 1, 0, 3)
    ).reshape(2, P, DO * 512)
    wb8 = np.ascontiguousarray(W_b.reshape(AO, P).T)     # [P, AO] f32
    v16 = np.ascontiguousarray(v_w[0].reshape(AO, P).T.astype(BF))
    maskb = (mask.astype(np.float32) - 1.0) * 50.0       # 0 kept / -50 masked

    in_maps = []
    for c in range(N_CORES):
        sl = slice(B_LOC * c, B_LOC * (c + 1))
        dech = np.ascontiguousarray(
            dec_hidden[sl].T.reshape(DO, P, B_LOC).transpose(1, 0, 2)
        ).reshape(P, DO * B_LOC).astype(BF)
        in_maps.append(
            {
                "encT": np.ascontiguousarray(encT[sl]),
                "encN": np.ascontiguousarray(encN[sl]),
                "w_encT": wencT,
                "w_decT": wdecT,
                "dec_hT": dech,
                "v_col": v16,
                "wb8": wb8,
                "maskb": np.ascontiguousarray(maskb[sl]),
            }
        )
    return in_maps


def kernel(dec_hidden, enc_outputs, mask, W_w, W_b, v_w):
    from concourse.bass_utils import run_bass_kernel_spmd

    assert enc_outputs.shape == (N_CORES * B_LOC, TX, E), enc_outputs.shape
    nc = _get_nc()
    in_maps = make_in_maps(dec_hidden, enc_outputs, mask, W_w, W_b, v_w)
    res = run_bass_kernel_spmd(nc, in_maps, list(range(N_CORES))).results
    context = np.concatenate([res[c]["context"] for c in range(N_CORES)], axis=0)
    alpha = np.concatenate([res[c]["alpha"] for c in range(N_CORES)], axis=0)
    return context, alpha


# revision 5
# speedup vs baseline: 1.0187x; 1.0187x over previous
"""Bahdanau attention kernel for Trainium2 (Bass/Tile), 8-core data-parallel.

Problem shapes: B=32, Tx=1024, enc_hid=dec_hid=attn=1024.

v3: bf16 + example-paired weight sharing.
  - All big matmul operands bf16 (validated: ctx rel 2.8e-3, tol 2e-2).
  - Examples processed in pairs sharing each stationary w_encT chunk
    across 2 consecutive matmuls (b0, b1 of the pair): measured HW shows
    a ~46ns weight-switch penalty per matmul (259ns vs 213ns streaming
    for N=512); same-weight back-to-back matmuls avoid it.
  - t-halves (nt) kept separate so each example's scores for half 0
    close at pair midpoint: softmax + context accumulate per half,
    hidden under later energy groups.
  - Scores/context accumulators for a pair are partition-packed into
    single PSUM banks (offsets 0/32/64/96). Only the first matmul to
    touch a bank uses start=True (bank-wide has_written clear); other
    accumulators' first matmuls rely on overwrite-where-clear.
  - exp via ACT with accum_out => softmax sum free; alpha row->column
    via tiny PE transposes (no DRAM bounce).

Math (per example b):
  dec_proj = W_dec @ dec_hidden[b]                 [attn]
  energy^T[a, t] = tanh(sum_e W_enc[a,e] enc[b,t,e] + dec_proj[a] + W_b[a])
  scores[t] = sum_a v[a] energy^T[a, t]
  alpha = softmax(scores + (mask-1)*50)
  context[e] = sum_t alpha[t] enc[b,t,e]
"""

from contextlib import ExitStack

import numpy as np
import ml_dtypes

import concourse.bass as bass
import concourse.tile as tile
from concourse import bacc, mybir
from concourse.masks import make_identity

F32 = mybir.dt.float32
BF16 = mybir.dt.bfloat16
AF = mybir.ActivationFunctionType
BF = ml_dtypes.bfloat16

P = 128
N_CORES = 8
B_LOC = 4            # examples per core
TX = 1024
E = 1024             # enc_hid
A = 1024             # attn
D = 1024             # dec_hid
EO = E // P
AO = A // P
TO = TX // P
DO = D // P
NT = 2               # 512-wide t-halves
ET = 2               # 512-wide e-halves


def build_nc():
    nc = bacc.Bacc(
        "TRN2", target_bir_lowering=False, debug=False, num_devices=N_CORES
    )
    encT_d = nc.dram_tensor("encT", [B_LOC, NT, P, EO * 512], BF16, kind="ExternalInput").ap()
    encN_d = nc.dram_tensor("encN", [B_LOC, P, TO * E], BF16, kind="ExternalInput").ap()
    wenc_d = nc.dram_tensor("w_encT", [AO, P, EO * P], BF16, kind="ExternalInput").ap()
    wdec_d = nc.dram_tensor("w_decT", [2, P, DO * 512], BF16, kind="ExternalInput").ap()
    dech_d = nc.dram_tensor("dec_hT", [P, DO * B_LOC], BF16, kind="ExternalInput").ap()
    v_d = nc.dram_tensor("v_col", [P, AO], BF16, kind="ExternalInput").ap()
    wb_d = nc.dram_tensor("wb8", [P, AO], F32, kind="ExternalInput").ap()
    maskb_d = nc.dram_tensor("maskb", [B_LOC, TX], F32, kind="ExternalInput").ap()
    ctx_out = nc.dram_tensor("context", [B_LOC, E], F32, kind="ExternalOutput").ap()
    alpha_out = nc.dram_tensor("alpha", [B_LOC, TX], F32, kind="ExternalOutput").ap()

    with tile.TileContext(nc) as tc, ExitStack() as ctx:
        const = ctx.enter_context(tc.tile_pool(name="const", bufs=1))
        encT_pool = ctx.enter_context(tc.tile_pool(name="encTp", bufs=4))
        encN_pool = ctx.enter_context(tc.tile_pool(name="encNp", bufs=3))
        en_pool = ctx.enter_context(tc.tile_pool(name="energy", bufs=6))
        rowp = ctx.enter_context(tc.tile_pool(name="rows", bufs=2))
        small = ctx.enter_context(tc.tile_pool(name="small", bufs=2))
        # PSUM bank budget (8): ep 3 + sc 2 + cx 1 + tiny 2
        ps_ep = ctx.enter_context(tc.tile_pool(name="ps_ep", bufs=3, space="PSUM"))
        ps_sc = ctx.enter_context(tc.tile_pool(name="ps_sc", bufs=2, space="PSUM"))
        ps_cx = ctx.enter_context(tc.tile_pool(name="ps_cx", bufs=1, space="PSUM"))
        ps_tiny = ctx.enter_context(tc.tile_pool(name="ps_tiny", bufs=2, space="PSUM"))

        wenc_sb = const.tile([P, AO, EO, P], BF16)
        wdec_sb = const.tile([P, 2, DO, 512], BF16)
        dech_sb = const.tile([P, DO, B_LOC], BF16)
        v_sb = const.tile([P, AO], BF16)
        wb_sb = const.tile([P, AO], F32)
        bias_sb = const.tile([P, AO, B_LOC], F32)
        ident4 = const.tile([B_LOC, B_LOC], F32)
        ident1 = const.tile([1, 1], F32)
        make_identity(nc, ident4[:])
        make_identity(nc, ident1[:])

        nc.gpsimd.dma_start(dech_sb[:].rearrange("p do b -> p (do b)"), dech_d[:])
        nc.gpsimd.dma_start(v_sb[:], v_d[:])
        nc.gpsimd.dma_start(wb_sb[:], wb_d[:])
        mask_rows = []
        for b in range(B_LOC):
            mr = small.tile([1, TX], F32, tag="mrow", bufs=B_LOC, name=f"mask{b}")
            nc.gpsimd.dma_start(mr[:], maskb_d[b : b + 1, :])
            mask_rows.append(mr)

        lanes3 = [nc.sync, nc.scalar, nc.gpsimd]
        lanes2 = [nc.sync, nc.gpsimd]
        li3 = [0]
        li2 = [0]

        def lane3():
            e = lanes3[li3[0] % 3]
            li3[0] += 1
            return e

        def lane2():
            e = lanes2[li2[0] % 2]
            li2[0] += 1
            return e

        def dma_chunks(dst2d, src2d, nchunks, lane_fn):
            n = src2d.shape[-1]
            step = n // nchunks
            for i in range(nchunks):
                lane_fn().dma_start(
                    dst2d[:, i * step : (i + 1) * step],
                    src2d[:, i * step : (i + 1) * step],
                )

        encT_tiles = {}
        encN_tiles = {}

        def alloc_encT(b):
            encT_tiles[b] = encT_pool.tile(
                [P, NT, EO, 512], BF16, tag="encT", name=f"encT{b}"
            )

        def alloc_encN(b):
            encN_tiles[b] = encN_pool.tile(
                [P, TO, E], BF16, tag="encN", name=f"encN{b}"
            )

        def encT_2d(b, nt):
            return encT_tiles[b][:, nt].rearrange("p eo j -> p (eo j)")

        def encN_2d(b):
            return encN_tiles[b][:].rearrange("p to e -> p (to e)")

        def wenc_2d(ao):
            return wenc_sb[:, ao].rearrange("p eo c -> p (eo c)")

        # ---- bulk loads in need order, chunked for multi-queue overlap
        for b in range(B_LOC):
            alloc_encT(b)
        alloc_encN(0)
        alloc_encN(1)
        alloc_encN(2)
        wdec0_2d = wdec_sb[:, 0].rearrange("p do j -> p (do j)")
        wdec1_2d = wdec_sb[:, 1].rearrange("p do j -> p (do j)")
        dma_chunks(wdec0_2d, wdec_d[0], 2, lane3)
        dma_chunks(wenc_2d(0), wenc_d[0], 1, lane3)
        dma_chunks(encT_2d(0, 0), encT_d[0, 0], 2, lane3)
        dma_chunks(encT_2d(1, 0), encT_d[1, 0], 2, lane3)
        dma_chunks(wenc_2d(1), wenc_d[1], 1, lane3)
        dma_chunks(wdec1_2d, wdec_d[1], 2, lane3)
        dma_chunks(wenc_2d(2), wenc_d[2], 1, lane3)
        dma_chunks(wenc_2d(3), wenc_d[3], 1, lane3)
        dma_chunks(encT_2d(0, 1), encT_d[0, 1], 2, lane3)
        dma_chunks(encT_2d(1, 1), encT_d[1, 1], 2, lane3)
        for ao in range(4, 8):
            dma_chunks(wenc_2d(ao), wenc_d[ao], 1, lane3)
        dma_chunks(encT_2d(2, 0), encT_d[2, 0], 1, lane3)
        dma_chunks(encT_2d(3, 0), encT_d[3, 0], 1, lane3)
        dma_chunks(encT_2d(2, 1), encT_d[2, 1], 1, lane3)
        dma_chunks(encT_2d(3, 1), encT_d[3, 1], 1, lane3)
        dma_chunks(encN_2d(0), encN_d[0], 2, lane3)
        dma_chunks(encN_2d(1), encN_d[1], 2, lane3)
        dma_chunks(encN_2d(2), encN_d[2], 2, lane3)

        # ---- per-pair / per-example state -------------------------------
        sc_banks = {}   # (pair, nt) -> [P, 512] PSUM tile; b0@p0, b1@p32
        cx_banks = {}   # pair -> [P, 512] PSUM tile; (bi, et)@p32*(2bi+et)
        expT_pss = {}   # pair -> [P, 16] f32 PSUM; cols bi*8 + nt*4 + i

        def sc_bank(p, nt):
            if (p, nt) not in sc_banks:
                sc_banks[(p, nt)] = ps_sc.tile(
                    [P, 512], F32, tag="sc", name=f"scb{p}_{nt}"
                )
            return sc_banks[(p, nt)]

        def cx_bank(p):
            if p not in cx_banks:
                cx_banks[p] = ps_cx.tile([P, 512], F32, tag="cx", name=f"cxb{p}")
            return cx_banks[p]

        def expT_ps(p):
            if p not in expT_pss:
                expT_pss[p] = ps_tiny.tile(
                    [P, 16], F32, tag="tiny", name=f"expTps{p}"
                )
            return expT_pss[p]

        class Ex:
            pass

        exs = {}

        def get_ex(b):
            if b in exs:
                return exs[b]
            s = Ex()
            s.msc = rowp.tile([1, TX], F32, tag="msc", name=f"msc{b}")
            s.expf = rowp.tile([1, TX], F32, tag="expf", name=f"expf{b}")
            s.s2 = small.tile([1, 2], F32, tag="s2", name=f"s2_{b}")
            s.expT = small.tile([P, TO], BF16, tag="expT", name=f"expT{b}")
            s.alpha_row = rowp.tile([1, TX], F32, tag="arow", name=f"alpha{b}")
            s.ctx_row = rowp.tile([1, E], F32, tag="crow", name=f"ctx{b}")
            exs[b] = s
            return s

        # ---- compute blocks ---------------------------------------------
        dp_row = rowp.tile([B_LOC, A], F32, tag="dprow", bufs=1)

        def dec_half(h):
            dp = ps_ep.tile([P, 512], F32, tag="ep", name=f"dp{h}")
            for do in range(DO):
                nc.tensor.matmul(
                    dp[:B_LOC, :],
                    lhsT=dech_sb[:, do],
                    rhs=wdec_sb[:, h, do],
                    start=(do == 0),
                    stop=(do == DO - 1),
                )
            nc.vector.tensor_copy(dp_row[:, h * 512 : (h + 1) * 512], dp[:B_LOC, :])
            for ao in range(4 * h, 4 * h + 4):
                tp = ps_tiny.tile([P, B_LOC], F32, tag="tiny", name=f"tp{ao}")
                nc.tensor.transpose(tp[:], dp_row[:, ao * P : (ao + 1) * P], ident4[:])
                nc.vector.tensor_scalar_add(bias_sb[:, ao], tp[:], wb_sb[:, ao : ao + 1])

        def eg_pair(p, nt, ao):
            # energy tiles for both pair members; each w_encT chunk loaded
            # once serves the b0 and b1 matmuls back to back.
            b0, b1 = 2 * p, 2 * p + 1
            scb = sc_bank(p, nt)
            ep0 = ps_ep.tile([P, 512], F32, tag="ep", name=f"ep{p}_{nt}_{ao}_0")
            ep1 = ps_ep.tile([P, 512], F32, tag="ep", name=f"ep{p}_{nt}_{ao}_1")
            for eo in range(EO):
                nc.tensor.matmul(
                    ep0[:], lhsT=wenc_sb[:, ao, eo], rhs=encT_tiles[b0][:, nt, eo],
                    start=(eo == 0), stop=(eo == EO - 1),
                )
                nc.tensor.matmul(
                    ep1[:], lhsT=wenc_sb[:, ao, eo], rhs=encT_tiles[b1][:, nt, eo],
                    start=(eo == 0), stop=(eo == EO - 1),
                )
            for bi, ep in ((0, ep0), (1, ep1)):
                b = 2 * p + bi
                en = en_pool.tile([P, 512], BF16, tag="energy", name=f"en{b}_{nt}_{ao}")
                nc.scalar.activation(
                    en[:], ep[:], AF.Tanh, bias=bias_sb[:, ao, b : b + 1]
                )
                nc.tensor.matmul(
                    scb[32 * bi : 32 * bi + 1, :],
                    lhsT=v_sb[:, ao : ao + 1],
                    rhs=en[:],
                    start=(ao == 0 and bi == 0),  # bank-wide clear, once
                    stop=(ao == AO - 1),
                    skip_group_check=True,
                    tile_position=(0, 32 * bi),
                )

        def half_block(b, nt):
            # masked scores -> exp(+sum) -> expT columns -> context MMs for
            # this half's t-chunks.
            p, bi = b // 2, b % 2
            s = get_ex(b)
            hs = slice(nt * 512, (nt + 1) * 512)
            nc.vector.tensor_add(
                s.msc[:, hs], sc_bank(p, nt)[32 * bi : 32 * bi + 1, :],
                mask_rows[b][:, hs],
            )
            nc.scalar.activation(
                s.expf[:, hs], s.msc[:, hs], AF.Exp,
                accum_out=s.s2[:, nt : nt + 1],
            )
            etp = expT_ps(p)
            col0 = bi * 8 + nt * 4
            for i in range(4):
                to = nt * 4 + i
                nc.tensor.transpose(
                    etp[:, col0 + i : col0 + i + 1],
                    s.expf[:, to * P : (to + 1) * P],
                    ident1[:],
                )
            nc.vector.tensor_copy(
                s.expT[:, nt * 4 : nt * 4 + 4], etp[:, col0 : col0 + 4]
            )
            cxb = cx_bank(p)
            for et in range(ET):
                row = 32 * (2 * bi + et)
                for i in range(4):
                    to = nt * 4 + i
                    nc.tensor.matmul(
                        cxb[row : row + 1, :],
                        lhsT=s.expT[:, to : to + 1],
                        rhs=encN_tiles[b][:, to, et * 512 : (et + 1) * 512],
                        start=(to == 0 and bi == 0 and et == 0),
                        stop=(to == TO - 1),
                        skip_group_check=True,
                        tile_position=(0, row),
                    )

        def fin(b):
            p, bi = b // 2, b % 2
            s = get_ex(b)
            ssum = small.tile([1, 1], F32, tag="ssum", name=f"ssum{b}")
            nc.vector.tensor_add(ssum[:], s.s2[:, 0:1], s.s2[:, 1:2])
            rsum = small.tile([1, 1], F32, tag="rsum", name=f"rsum{b}")
            nc.vector.reciprocal(rsum[:], ssum[:])
            nc.vector.tensor_scalar_mul(s.alpha_row[:], s.expf[:], rsum[:])
            nc.sync.dma_start(alpha_out[b : b + 1, :], s.alpha_row[:])
            cxb = cx_bank(p)
            for et in range(ET):
                row = 32 * (2 * bi + et)
                nc.vector.tensor_scalar_mul(
                    s.ctx_row[:, et * 512 : (et + 1) * 512],
                    cxb[row : row + 1, :],
                    rsum[:],
                )
            nc.sync.dma_start(ctx_out[b : b + 1, :], s.ctx_row[:])

        # ---- software-pipelined program ---------------------------------
        dec_half(0)
        eg_pair(0, 0, 0)
        dec_half(1)
        for ao in range(1, 8):
            eg_pair(0, 0, ao)
        eg_pair(0, 1, 0)
        eg_pair(0, 1, 1)
        half_block(0, 0)
        half_block(1, 0)
        for ao in range(2, 8):
            eg_pair(0, 1, ao)
        eg_pair(1, 0, 0)
        eg_pair(1, 0, 1)
        half_block(0, 1)
        fin(0)
        half_block(1, 1)
        fin(1)
        for ao in range(2, 8):
            eg_pair(1, 0, ao)
        # last encN is slot-gated: keep its enqueue off the scalar queue
        alloc_encN(3)
        dma_chunks(encN_2d(3), encN_d[3], 2, lane2)
        eg_pair(1, 1, 0)
        eg_pair(1, 1, 1)
        half_block(2, 0)
        half_block(3, 0)
        for ao in range(2, 8):
            eg_pair(1, 1, ao)
        half_block(2, 1)
        fin(2)
        half_block(3, 1)
        fin(3)

    nc.compile()
    return nc


_NC = None


def _get_nc():
    global _NC
    if _NC is None:
        _NC = build_nc()
    return _NC


def make_in_maps(dec_hidden, enc_outputs, mask, W_w, W_b, v_w):
    dec_hidden = np.asarray(dec_hidden, np.float32)
    enc_outputs = np.asarray(enc_outputs, np.float32)
    mask = np.asarray(mask)
    W_w = np.asarray(W_w, np.float32)
    W_b = np.asarray(W_b, np.float32)
    v_w = np.asarray(v_w, np.float32)

    enc16 = enc_outputs.astype(BF)                       # [B, TX, E]
    # encT[b, nt, p, eo*512+j] = enc[b, nt*512+j, eo*128+p]
    x = enc16.reshape(-1, NT, 512, EO, P)
    encT = np.ascontiguousarray(x.transpose(0, 1, 4, 3, 2)).reshape(
        -1, NT, P, EO * 512
    )
    # encN[b, p, to*1024+e] = enc[b, to*128+p, e]
    y = enc16.reshape(-1, TO, P, E)
    encN = np.ascontiguousarray(y.transpose(0, 2, 1, 3)).reshape(-1, P, TO * E)

    W_enc = W_w[:, D:].astype(BF)                        # [A, E]
    wencT = np.ascontiguousarray(
        W_enc.reshape(AO, P, EO, P).transpose(0, 3, 2, 1)
    ).reshape(AO, P, EO * P)
    W_dec = W_w[:, :D].astype(BF)                        # [A, D]
    wdecT = np.ascontiguousarray(
        W_dec.T.reshape(DO, P, 2, 512).transpose(2, 1, 0, 3)
    ).reshape(2, P, DO * 512)
    wb8 = np.ascontiguousarray(W_b.reshape(AO, P).T)     # [P, AO] f32
    v16 = np.ascontiguousarray(v_w[0].reshape(AO, P).T.astype(BF))
    maskb = (mask.astype(np.float32) - 1.0) * 50.0       # 0 kept / -50 masked

    in_maps = []
    for c in range(N_CORES):
        sl = slice(B_LOC * c, B_LOC * (c + 1))
        dech = np.ascontiguousarray(
            dec_hidden[sl].T.reshape(DO, P, B_LOC).transpose(1, 0, 2)
        ).reshape(P, DO * B_LOC).astype(BF)
        in_maps.append(
            {
                "encT": np.ascontiguousarray(encT[sl]),
                "encN": np.ascontiguousarray(encN[sl]),
                "w_encT": wencT,
                "w_decT": wdecT,
                "dec_hT": dech,
                "v_col": v16,
                "wb8": wb8,
                "maskb": np.ascontiguousarray(maskb[sl]),
            }
        )
    return in_maps


def kernel(dec_hidden, enc_outputs, mask, W_w, W_b, v_w):
    from concourse.bass_utils import run_bass_kernel_spmd

    assert enc_outputs.shape == (N_CORES * B_LOC, TX, E), enc_outputs.shape
    nc = _get_nc()
    in_maps = make_in_maps(dec_hidden, enc_outputs, mask, W_w, W_b, v_w)
    res = run_bass_kernel_spmd(nc, in_maps, list(range(N_CORES))).results
    context = np.concatenate([res[c]["context"] for c in range(N_CORES)], axis=0)
    alpha = np.concatenate([res[c]["alpha"] for c in range(N_CORES)], axis=0)
    return context, alpha


# revision 7
# speedup vs baseline: 1.1661x; 1.1447x over previous
"""Bahdanau attention kernel for Trainium2 (Bass/Tile), 8-core data-parallel.

Problem shapes: B=32, Tx=1024, enc_hid=dec_hid=attn=1024.

v4: bf16 + nt-paired weight sharing.
  - All big matmul operands bf16 (validated: ctx rel ~2.8e-3, tol 2e-2).
  - Host pre-tiles every tensor so each DMA is a contiguous 2D slab.
  - Energy groups pair the two 512-wide t-halves: each w_encT chunk is
    loaded once and serves the nt0/nt1 matmuls back to back (measured
    ~46ns weight-switch penalty per matmul otherwise), and consecutive
    matmuls alternate PSUM banks.
  - exp via ACT accum_out => softmax sum free; alpha row->column via
    tiny PE transposes (no DRAM bounce).
  - PE stream software-pipelined: the next example's first two energy
    groups are emitted before the current example's softmax tail.

Math (per example b):
  dec_proj = W_dec @ dec_hidden[b]                 [attn]
  energy^T[a, t] = tanh(sum_e W_enc[a,e] enc[b,t,e] + dec_proj[a] + W_b[a])
  scores[t] = sum_a v[a] energy^T[a, t]
  alpha = softmax(scores + (mask-1)*50)
  context[e] = sum_t alpha[t] enc[b,t,e]
"""

from contextlib import ExitStack

import numpy as np
import ml_dtypes

import concourse.bass as bass
import concourse.tile as tile
from concourse import bacc, mybir
from concourse.masks import make_identity

F32 = mybir.dt.float32
BF16 = mybir.dt.bfloat16
AF = mybir.ActivationFunctionType
BF = ml_dtypes.bfloat16

P = 128
N_CORES = 8
B_LOC = 4            # examples per core
TX = 1024
E = 1024             # enc_hid
A = 1024             # attn
D = 1024             # dec_hid
EO = E // P
AO = A // P
TO = TX // P
DO = D // P
NT = 2               # 512-wide t-halves
ET = 2               # 512-wide e-halves


def build_nc():
    nc = bacc.Bacc(
        "TRN2", target_bir_lowering=False, debug=False, num_devices=N_CORES
    )
    encT_d = nc.dram_tensor("encT", [B_LOC, NT, P, EO * 512], BF16, kind="ExternalInput").ap()
    encN_d = nc.dram_tensor("encN", [B_LOC, P, TO * E], BF16, kind="ExternalInput").ap()
    wenc_d = nc.dram_tensor("w_encT", [AO, P, EO * P], BF16, kind="ExternalInput").ap()
    wdec_d = nc.dram_tensor("w_decT", [2, P, DO * 512], BF16, kind="ExternalInput").ap()
    dech_d = nc.dram_tensor("dec_hT", [P, DO * B_LOC], BF16, kind="ExternalInput").ap()
    v_d = nc.dram_tensor("v_col", [P, AO], BF16, kind="ExternalInput").ap()
    wb_d = nc.dram_tensor("wb8", [P, AO], F32, kind="ExternalInput").ap()
    maskb_d = nc.dram_tensor("maskb", [B_LOC, TX], F32, kind="ExternalInput").ap()
    ctx_out = nc.dram_tensor("context", [B_LOC, E], F32, kind="ExternalOutput").ap()
    alpha_out = nc.dram_tensor("alpha", [B_LOC, TX], F32, kind="ExternalOutput").ap()

    with tile.TileContext(nc) as tc, ExitStack() as ctx:
        const = ctx.enter_context(tc.tile_pool(name="const", bufs=1))
        encT_pool = ctx.enter_context(tc.tile_pool(name="encTp", bufs=3))
        encN_pool = ctx.enter_context(tc.tile_pool(name="encNp", bufs=3))
        en_pool = ctx.enter_context(tc.tile_pool(name="energy", bufs=6))
        rowp = ctx.enter_context(tc.tile_pool(name="rows", bufs=2))
        small = ctx.enter_context(tc.tile_pool(name="small", bufs=2))
        ps_ep = ctx.enter_context(tc.tile_pool(name="ps_ep", bufs=3, space="PSUM"))
        ps_sc = ctx.enter_context(tc.tile_pool(name="ps_sc", bufs=2, space="PSUM"))
        ps_cx = ctx.enter_context(tc.tile_pool(name="ps_cx", bufs=2, space="PSUM"))
        ps_tiny = ctx.enter_context(tc.tile_pool(name="ps_tiny", bufs=1, space="PSUM"))

        wenc_sb = const.tile([P, AO, EO, P], BF16)
        wdec_sb = const.tile([P, 2, DO, 512], BF16)
        dech_sb = const.tile([P, DO, B_LOC], BF16)
        v_sb = const.tile([P, AO], BF16)
        wb_sb = const.tile([P, AO], F32)
        bias_sb = const.tile([P, AO, B_LOC], F32)
        ident4 = const.tile([B_LOC, B_LOC], F32)
        ident1 = const.tile([1, 1], F32)
        make_identity(nc, ident4[:])
        make_identity(nc, ident1[:])

        nc.gpsimd.dma_start(dech_sb[:].rearrange("p do b -> p (do b)"), dech_d[:])
        nc.gpsimd.dma_start(v_sb[:], v_d[:])
        nc.gpsimd.dma_start(wb_sb[:], wb_d[:])
        mask_rows = []
        for b in range(B_LOC):
            mr = small.tile([1, TX], F32, tag="mrow", bufs=B_LOC, name=f"mask{b}")
            nc.gpsimd.dma_start(mr[:], maskb_d[b : b + 1, :])
            mask_rows.append(mr)

        lanes3 = [nc.sync, nc.scalar, nc.gpsimd]
        lanes2 = [nc.sync, nc.gpsimd]
        li3 = [0]
        li2 = [0]

        def lane3():
            e = lanes3[li3[0] % 3]
            li3[0] += 1
            return e

        def lane2():
            e = lanes2[li2[0] % 2]
            li2[0] += 1
            return e

        def dma_chunks(dst2d, src2d, nchunks, lane_fn):
            n = src2d.shape[-1]
            step = n // nchunks
            for i in range(nchunks):
                lane_fn().dma_start(
                    dst2d[:, i * step : (i + 1) * step],
                    src2d[:, i * step : (i + 1) * step],
                )

        encT_tiles = {}
        encN_tiles = {}

        def alloc_encT(b):
            encT_tiles[b] = encT_pool.tile(
                [P, NT, EO, 512], BF16, tag="encT", name=f"encT{b}"
            )

        def alloc_encN(b):
            encN_tiles[b] = encN_pool.tile(
                [P, TO, E], BF16, tag="encN", name=f"encN{b}"
            )

        def encT_2d(b, nt):
            return encT_tiles[b][:, nt].rearrange("p eo j -> p (eo j)")

        def encN_2d(b):
            return encN_tiles[b][:].rearrange("p to e -> p (to e)")

        def wenc_2d(ao):
            return wenc_sb[:, ao].rearrange("p eo c -> p (eo c)")

        # ---- bulk loads in need order, chunked across the 3 queues ------
        alloc_encT(0)
        alloc_encT(1)
        alloc_encT(2)
        alloc_encN(0)
        alloc_encN(1)
        alloc_encN(2)
        wdec0_2d = wdec_sb[:, 0].rearrange("p do j -> p (do j)")
        wdec1_2d = wdec_sb[:, 1].rearrange("p do j -> p (do j)")
        dma_chunks(wdec0_2d, wdec_d[0], 2, lane3)
        dma_chunks(wenc_2d(0), wenc_d[0], 1, lane3)
        dma_chunks(encT_2d(0, 0), encT_d[0, 0], 2, lane3)
        dma_chunks(encT_2d(0, 1), encT_d[0, 1], 2, lane3)
        dma_chunks(wdec1_2d, wdec_d[1], 2, lane3)
        dma_chunks(wenc_2d(1), wenc_d[1], 1, lane3)
        dma_chunks(wenc_2d(2), wenc_d[2], 1, lane3)
        dma_chunks(wenc_2d(3), wenc_d[3], 1, lane3)
        dma_chunks(encT_2d(1, 0), encT_d[1, 0], 2, lane3)
        dma_chunks(encT_2d(1, 1), encT_d[1, 1], 2, lane3)
        for ao in range(4, 8):
            dma_chunks(wenc_2d(ao), wenc_d[ao], 1, lane3)
        dma_chunks(encN_2d(0), encN_d[0], 2, lane3)
        dma_chunks(encT_2d(2, 0), encT_d[2, 0], 1, lane3)
        dma_chunks(encT_2d(2, 1), encT_d[2, 1], 1, lane3)
        dma_chunks(encN_2d(1), encN_d[1], 2, lane3)
        dma_chunks(encN_2d(2), encN_d[2], 2, lane3)

        # ---- per-example state ------------------------------------------
        class Ex:
            pass

        exs = {}

        def get_ex(b):
            if b in exs:
                return exs[b]
            s = Ex()
            s.sc = [
                ps_sc.tile([1, 512], F32, tag="sc", name=f"sc{b}_{nt}")
                for nt in range(NT)
            ]
            s.msc = rowp.tile([1, TX], F32, tag="msc", name=f"msc{b}")
            s.expf = rowp.tile([1, TX], F32, tag="expf", name=f"expf{b}")
            s.s2 = small.tile([1, 2], F32, tag="s2", name=f"s2_{b}")
            s.expT_ps = ps_tiny.tile([P, TO], F32, tag="tiny", name=f"expTps{b}")
            s.expT = small.tile([P, TO], BF16, tag="expT", name=f"expT{b}")
            s.cx = [
                ps_cx.tile([1, 512], F32, tag="cx", name=f"cx{b}_{et}")
                for et in range(ET)
            ]
            s.alpha_row = rowp.tile([1, TX], F32, tag="arow", name=f"alpha{b}")
            s.ctx_row = rowp.tile([1, E], F32, tag="crow", name=f"ctx{b}")
            exs[b] = s
            return s

        # ---- compute blocks ---------------------------------------------
        dp_row = rowp.tile([B_LOC, A], F32, tag="dprow", bufs=1)

        def dec_half(h):
            dp = ps_ep.tile([P, 512], F32, tag="ep", name=f"dp{h}")
            for do in range(DO):
                nc.tensor.matmul(
                    dp[:B_LOC, :],
                    lhsT=dech_sb[:, do],
                    rhs=wdec_sb[:, h, do],
                    start=(do == 0),
                    stop=(do == DO - 1),
                )
            nc.vector.tensor_copy(dp_row[:, h * 512 : (h + 1) * 512], dp[:B_LOC, :])
            for ao in range(4 * h, 4 * h + 4):
                tp = ps_tiny.tile([P, B_LOC], F32, tag="tiny", name=f"tp{ao}")
                nc.tensor.transpose(tp[:], dp_row[:, ao * P : (ao + 1) * P], ident4[:])
                nc.vector.tensor_scalar_add(bias_sb[:, ao], tp[:], wb_sb[:, ao : ao + 1])

        def e_group(b, ao):
            # both t-halves' energy tiles for this a-chunk; each w_encT
            # chunk loaded once serves the nt0/nt1 matmuls back to back
            s = get_ex(b)
            ep0 = ps_ep.tile([P, 512], F32, tag="ep", name=f"ep{b}_{ao}_0")
            ep1 = ps_ep.tile([P, 512], F32, tag="ep", name=f"ep{b}_{ao}_1")
            for eo in range(EO):
                nc.tensor.matmul(
                    ep0[:], lhsT=wenc_sb[:, ao, eo], rhs=encT_tiles[b][:, 0, eo],
                    start=(eo == 0), stop=(eo == EO - 1),
                )
                nc.tensor.matmul(
                    ep1[:], lhsT=wenc_sb[:, ao, eo], rhs=encT_tiles[b][:, 1, eo],
                    start=(eo == 0), stop=(eo == EO - 1),
                )
            for nt, ep in ((0, ep0), (1, ep1)):
                en = en_pool.tile([P, 512], BF16, tag="energy", name=f"en{b}_{nt}_{ao}")
                nc.scalar.activation(
                    en[:], ep[:], AF.Tanh, bias=bias_sb[:, ao, b : b + 1]
                )
                nc.tensor.matmul(
                    s.sc[nt][:],
                    lhsT=v_sb[:, ao : ao + 1],
                    rhs=en[:],
                    start=(ao == 0),
                    stop=(ao == AO - 1),
                )

        def half_block(b, nt):
            s = get_ex(b)
            hs = slice(nt * 512, (nt + 1) * 512)
            nc.vector.tensor_add(s.msc[:, hs], s.sc[nt][:], mask_rows[b][:, hs])
            nc.scalar.activation(
                s.expf[:, hs], s.msc[:, hs], AF.Exp,
                accum_out=s.s2[:, nt : nt + 1],
            )
            for i in range(4):
                to = nt * 4 + i
                nc.tensor.transpose(
                    s.expT_ps[:, to : to + 1],
                    s.expf[:, to * P : (to + 1) * P],
                    ident1[:],
                )
            nc.vector.tensor_copy(
                s.expT[:, nt * 4 : nt * 4 + 4], s.expT_ps[:, nt * 4 : nt * 4 + 4]
            )
            for et in range(ET):
                for i in range(4):
                    to = nt * 4 + i
                    nc.tensor.matmul(
                        s.cx[et][:],
                        lhsT=s.expT[:, to : to + 1],
                        rhs=encN_tiles[b][:, to, et * 512 : (et + 1) * 512],
                        start=(to == 0),
                        stop=(to == TO - 1),
                    )

        def fin(b):
            s = get_ex(b)
            ssum = small.tile([1, 1], F32, tag="ssum", name=f"ssum{b}")
            nc.vector.tensor_add(ssum[:], s.s2[:, 0:1], s.s2[:, 1:2])
            rsum = small.tile([1, 1], F32, tag="rsum", name=f"rsum{b}")
            nc.vector.reciprocal(rsum[:], ssum[:])
            nc.vector.tensor_scalar_mul(s.alpha_row[:], s.expf[:], rsum[:])
            nc.sync.dma_start(alpha_out[b : b + 1, :], s.alpha_row[:])
            for et in range(ET):
                nc.vector.tensor_scalar_mul(
                    s.ctx_row[:, et * 512 : (et + 1) * 512], s.cx[et][:], rsum[:]
                )
            nc.sync.dma_start(ctx_out[b : b + 1, :], s.ctx_row[:])

        # ---- software-pipelined program ---------------------------------
        dec_half(0)
        e_group(0, 0)
        dec_half(1)
        for ao in range(1, 8):
            e_group(0, ao)

        for b in range(B_LOC):
            nb = b + 1
            if b == 1:
                alloc_encT(3)
                dma_chunks(encT_2d(3, 0), encT_d[3, 0], 1, lane2)
                dma_chunks(encT_2d(3, 1), encT_d[3, 1], 1, lane2)
                alloc_encN(3)
                dma_chunks(encN_2d(3), encN_d[3], 2, lane2)
            if nb < B_LOC:
                # cover this example's softmax tail with the next
                # example's first energy groups
                e_group(nb, 0)
                e_group(nb, 1)
            half_block(b, 0)
            half_block(b, 1)
            fin(b)
            if nb < B_LOC:
                for ao in range(2, 8):
                    e_group(nb, ao)

    nc.compile()
    return nc


_NC = None


def _get_nc():
    global _NC
    if _NC is None:
        _NC = build_nc()
    return _NC


def make_in_maps(dec_hidden, enc_outputs, mask, W_w, W_b, v_w):
    dec_hidden = np.asarray(dec_hidden, np.float32)
    enc_outputs = np.asarray(enc_outputs, np.float32)
    mask = np.asarray(mask)
    W_w = np.asarray(W_w, np.float32)
    W_b = np.asarray(W_b, np.float32)
    v_w = np.asarray(v_w, np.float32)

    enc16 = enc_outputs.astype(BF)                       # [B, TX, E]
    # encT[b, nt, p, eo*512+j] = enc[b, nt*512+j, eo*128+p]
    x = enc16.reshape(-1, NT, 512, EO, P)
    encT = np.ascontiguousarray(x.transpose(0, 1, 4, 3, 2)).reshape(
        -1, NT, P, EO * 512
    )
    # encN[b, p, to*1024+e] = enc[b, to*128+p, e]
    y = enc16.reshape(-1, TO, P, E)
    encN = np.ascontiguousarray(y.transpose(0, 2, 1, 3)).reshape(-1, P, TO * E)

    W_enc = W_w[:, D:].astype(BF)                        # [A, E]
    wencT = np.ascontiguousarray(
        W_enc.reshape(AO, P, EO, P).transpose(0, 3, 2, 1)
    ).reshape(AO, P, EO * P)
    W_dec = W_w[:, :D].astype(BF)                        # [A, D]
    wdecT = np.ascontiguousarray(
        W_dec.T.reshape(DO, P, 2, 512).transpose(2, 1, 0, 3)
    ).reshape(2, P, DO * 512)
    wb8 = np.ascontiguousarray(W_b.reshape(AO, P).T)     # [P, AO] f32
    v16 = np.ascontiguousarray(v_w[0].reshape(AO, P).T.astype(BF))
    maskb = (mask.astype(np.float32) - 1.0) * 50.0       # 0 kept / -50 masked

    in_maps = []
    for c in range(N_CORES):
        sl = slice(B_LOC * c, B_LOC * (c + 1))
        dech = np.ascontiguousarray(
            dec_hidden[sl].T.reshape(DO, P, B_LOC).transpose(1, 0, 2)
        ).reshape(P, DO * B_LOC).astype(BF)
        in_maps.append(
            {
                "encT": np.ascontiguousarray(encT[sl]),
                "encN": np.ascontiguousarray(encN[sl]),
                "w_encT": wencT,
                "w_decT": wdecT,
                "dec_hT": dech,
                "v_col": v16,
                "wb8": wb8,
                "maskb": np.ascontiguousarray(maskb[sl]),
            }
        )
    return in_maps


def kernel(dec_hidden, enc_outputs, mask, W_w, W_b, v_w):
    from concourse.bass_utils import run_bass_kernel_spmd

    assert enc_outputs.shape == (N_CORES * B_LOC, TX, E), enc_outputs.shape
    nc = _get_nc()
    in_maps = make_in_maps(dec_hidden, enc_outputs, mask, W_w, W_b, v_w)
    res = run_bass_kernel_spmd(nc, in_maps, list(range(N_CORES))).results
    context = np.concatenate([res[c]["context"] for c in range(N_CORES)], axis=0)
    alpha = np.concatenate([res[c]["alpha"] for c in range(N_CORES)], axis=0)
    return context, alpha
